# revision 10
# baseline (speedup 1.0000x reference)
"""GCNConv Trainium2 kernel: out = segment_sum(features[src], dst) @ W + b.

Strategy (8 NeuronCores, graph partitioned by destination node):
  - Host: partition the dst nodes across 8 cores (degree-balanced snake),
    49 slots of <=128 dst nodes per core.  The SWDGE descriptor rate is the
    HW bottleneck, so source-feature rows are PAIRED: each core gets two
    privately permuted copies of the feature table laid out as 25000
    512-byte pair-rows; a slot-local greedy matching places co-occurring
    sources adjacently so one dma_gather descriptor (512B, one pair) serves
    two edges.  Residual edges gather a pair and use one half.
  - Device (per core): dma_gather pair descriptors (elem 256 bf16) in
    consumption order across 4 SWDGE queues; per gathered 128-desc chunk,
    matmuls with host-precomputed one-hot blocks (streamed over the idle
    hardware DMA queues) accumulate agg.T per dst slot in PSUM; then
    out.T = W.T @ agg.T and a fused bias-add; DMA out.T tiles to DRAM.
  - Host: scatter per-core tile outputs back to [50000, 128].
"""

import os
import sys

for _p in ("/opt/trn_rl_repo",):
    if _p not in sys.path and os.path.isdir(_p):
        sys.path.insert(0, _p)

import numpy as np
import ml_dtypes

P = 128
N_NODES = 50000
N_EDGES = 640000
D = 128
NCORES = 8
NPAIR = N_NODES // 2          # 25000 pair-rows per table copy
NSLOT = (N_NODES // NCORES + P - 1) // P   # 49
GCHUNK = 8                    # chunks (of 128 descs) per dma_gather call
TAILCH = 2                    # chunk count of the final small groups
NQUEUES = 4
GBUFS = 8
SINGLE_PACKET = False

BF16 = ml_dtypes.bfloat16
STREAMS = ("A", "B", "S")


# ---------------------------------------------------------------- host plan

def _assign_nodes(deg):
    """Degree-balanced snake deal of nodes to (core, slot, pos)."""
    order = np.argsort(-deg, kind="stable")
    snake = np.concatenate([np.arange(NCORES), np.arange(NCORES)[::-1]])
    core_of = np.empty(N_NODES, np.int32)
    core_of[order] = snake[np.arange(N_NODES) % (2 * NCORES)]
    slot_of = np.empty(N_NODES, np.int32)
    pos_of = np.empty(N_NODES, np.int32)
    node_lists = []
    ssnake = np.concatenate([np.arange(NSLOT), np.arange(NSLOT)[::-1]])
    for c in range(NCORES):
        nodes = order[core_of[order] == c]          # degree-sorted
        s = ssnake[np.arange(len(nodes)) % (2 * NSLOT)]
        # position within slot = running count per slot
        pos = np.zeros(len(nodes), np.int64)
        cnt = np.zeros(NSLOT, np.int64)
        for i, sl in enumerate(s):                   # 6250 per core
            pos[i] = cnt[sl]
            cnt[sl] += 1
        assert cnt.max() <= P
        slot_of[nodes] = s
        pos_of[nodes] = pos
        nl = [nodes[s == sl] for sl in range(NSLOT)]
        nl = [n[np.argsort(pos[s == sl], kind="stable")] for sl, n in enumerate(nl)]
        node_lists.append(nl)
    return core_of, slot_of, pos_of, node_lists


def _match_core(src_c, dpos_c, eslot_c):
    """Two-pass slot-local greedy pairing for one core.

    Nodes co-occurring in a slot are paired (placed adjacently in the table)
    so one 512B descriptor serves both edges; unmatched nodes are bucketed by
    their next other-occurrence slot so one pairing tends to cover two slots.
    Returns per-slot pair/single edge lists and the two permutations:
      pairsA[s] -> list of (g, ud, vd);  pairsB[s];  singE[s]/singO[s] ->
      list of (g, d) referencing copy-A pair g (even/odd half);
      posA/posB -> node -> row position in each copy.
    """
    nslot = NSLOT
    occ_map = {}   # u -> {slot: [dst positions]}
    for u, dp, s in zip(src_c.tolist(), dpos_c.tolist(), eslot_c.tolist()):
        occ_map.setdefault(u, {}).setdefault(s, []).append(dp)

    sizes = np.bincount(eslot_c, minlength=nslot)
    order = np.argsort(-sizes, kind="stable")

    posA = np.full(N_NODES, -1, np.int64)
    posB = np.full(N_NODES, -1, np.int64)
    placedA = np.full(N_NODES, -1, np.int64)
    placedB = np.full(N_NODES, -1, np.int64)
    pairsA = [[] for _ in range(nslot)]
    pairsB = [[] for _ in range(nslot)]

    def run_pass(pos, placed, pairs_out, occ_in, gctr):
        residual = {}
        processed = set()
        for s in order:
            processed.add(s)
            left = {u: list(sl[s]) for u, sl in occ_in.items()
                    if s in sl and sl[s]}
            # existing partners both present in this slot
            for u in list(left):
                pp = int(pos[u])
                if pp < 0 or pp % 2 == 1:
                    continue
                v = int(placed[pp + 1]) if pp + 1 < N_NODES else -1
                if v < 0 or v not in left:
                    continue
                ku, kv = left[u], left[v]
                k = min(len(ku), len(kv))
                g = pp // 2
                for _ in range(k):
                    pairs_out[s].append((g, ku.pop(), kv.pop()))
            # bucket unmatched nodes by their next unprocessed other-slot
            un = [u for u in left if pos[u] < 0 and left[u]]
            buckets = {}
            rest = []
            for u in un:
                others = [t for t in occ_in[u]
                          if t != s and t not in processed and occ_in[u][t]]
                if others:
                    buckets.setdefault(min(others), []).append(u)
                else:
                    rest.append(u)

            def wed(u, v):
                g = gctr[0]
                gctr[0] += 1
                pos[u], pos[v] = 2 * g, 2 * g + 1
                placed[2 * g], placed[2 * g + 1] = u, v
                ku, kv = left[u], left[v]
                k = min(len(ku), len(kv))
                for _ in range(k):
                    pairs_out[s].append((g, ku.pop(), kv.pop()))

            for key in sorted(buckets):
                bs = buckets[key]
                for i in range(0, len(bs) - 1, 2):
                    wed(bs[i], bs[i + 1])
                if len(bs) % 2:
                    rest.append(bs[-1])
            for i in range(0, len(rest) - 1, 2):
                wed(rest[i], rest[i + 1])
            for u, ds in left.items():
                if ds:
                    residual.setdefault(u, {}).setdefault(s, []).extend(ds)
        return residual

    gA = [0]
    res1 = run_pass(posA, placedA, pairsA, occ_map, gA)
    gB = [0]
    res2 = run_pass(posB, placedB, pairsB, res1, gB)

    # place every remaining node
    for pos, plc in ((posA, placedA), (posB, placedB)):
        rem = np.where(pos < 0)[0]
        free = np.where(plc < 0)[0]
        assert len(rem) == len(free)
        pos[rem] = free
        plc[free] = rem

    singE = [[] for _ in range(nslot)]
    singO = [[] for _ in range(nslot)]
    for u, sl in res2.items():
        pa = int(posA[u])
        tgt = singE if pa % 2 == 0 else singO
        for s, ds in sl.items():
            for dp in ds:
                tgt[s].append((pa // 2, dp))
    return pairsA, pairsB, singE, singO, posA, posB


def plan(src, dst):
    src = np.asarray(src).astype(np.int64)
    dst = np.asarray(dst).astype(np.int64)
    deg = np.bincount(dst, minlength=N_NODES)
    core_of, slot_of, pos_of, node_lists = _assign_nodes(deg)

    per_core = []
    for c in range(NCORES):
        m = core_of[dst] == c
        per_core.append(_match_core(src[m], pos_of[dst[m]], slot_of[dst[m]]))

    # shared per-slot stream lengths (max over cores)
    def seglen(idx):
        return np.array([[len(pc[idx][s]) for s in range(NSLOT)]
                         for pc in per_core]).max(axis=0)

    LA, LB, LSE, LSO = seglen(0), seglen(1), seglen(2), seglen(3)
    for s in range(NSLOT):
        if LA[s] + LB[s] + LSE[s] + LSO[s] == 0:
            LSE[s] = 1
    LS = LSE + LSO
    startA = np.concatenate([[0], np.cumsum(LA)])
    startB = np.concatenate([[0], np.cumsum(LB)])
    startS = np.concatenate([[0], np.cumsum(LS)])
    KA = -(-int(startA[-1]) // P)
    KB = -(-int(startB[-1]) // P)
    KS = -(-int(startS[-1]) // P)

    # shared matmul schedule: per slot, list of (stream, chunk, win, ohcol)
    mms = []
    ohcol = 0
    for s in range(NSLOT):
        lst = []
        for name, st, ln in (("A", startA, LA), ("B", startB, LB)):
            if ln[s] == 0:
                continue
            c0, c1 = int(st[s]) // P, (int(st[s]) + int(ln[s]) - 1) // P
            for j in range(c0, c1 + 1):
                for win in (0, 1):
                    lst.append((name, j, win, ohcol))
                    ohcol += 1
        sse = int(startS[s])
        for win, ln in ((0, int(LSE[s])), (1, int(LSO[s]))):
            if ln == 0:
                sse += ln
                continue
            c0, c1 = sse // P, (sse + ln - 1) // P
            for j in range(c0, c1 + 1):
                lst.append(("S", j, win, ohcol))
                ohcol += 1
            sse += ln
        mms.append(lst)
    OHC = ohcol * P

    # per-core desc index arrays + one-hot matrix
    idxs = {"A": np.zeros((NCORES, KA * P), np.int16),
            "B": np.zeros((NCORES, KB * P), np.int16),
            "S": np.zeros((NCORES, KS * P), np.int16)}
    # ohm layout: [128 desc-position rows, OHC cols] = per-matmul 128-col
    # one-hot blocks; uint8 counts (cast to bf16 at upload)
    ohms = [np.zeros((P, OHC), np.uint8) for _ in range(NCORES)]

    for c in range(NCORES):
        pairsA, pairsB, singE, singO, posA, posB = per_core[c]
        for name, st, seglists in (("A", startA, pairsA), ("B", startB, pairsB)):
            arr = idxs[name][c]
            for s in range(NSLOT):
                base = int(st[s])
                for i, (g, ud, vd) in enumerate(seglists[s]):
                    arr[base + i] = g
        arr = idxs["S"][c]
        for s in range(NSLOT):
            base = int(startS[s])
            for i, (g, dp) in enumerate(singE[s]):
                arr[base + i] = g
            base += int(LSE[s])
            for i, (g, dp) in enumerate(singO[s]):
                arr[base + i] = g
        # one-hots
        oh = ohms[c]
        for s in range(NSLOT):
            # map (stream, chunk, win) -> ohcol for this slot
            cmap = {(nm, j, w): col for (nm, j, w, col) in mms[s]}
            for name, st, seglists, winof in (
                    ("A", startA, pairsA, None), ("B", startB, pairsB, None)):
                base = int(st[s])
                for i, (g, ud, vd) in enumerate(seglists[s]):
                    p = base + i
                    j, r = p // P, p % P
                    oh[r, cmap[(name, j, 0)] * P + ud] += 1
                    oh[r, cmap[(name, j, 1)] * P + vd] += 1
            base = int(startS[s])
            for win, seg in ((0, singE[s]), (1, singO[s])):
                for i, (g, dp) in enumerate(seg):
                    p = base + i
                    j, r = p // P, p % P
                    oh[r, cmap[("S", j, win)] * P + dp] += 1
                base += int(LSE[s]) if win == 0 else 0
    return {
        "node_lists": node_lists,
        "per_core": per_core,
        "LA": LA, "LB": LB, "LSE": LSE, "LSO": LSO,
        "KA": KA, "KB": KB, "KS": KS,
        "mms": mms, "OHC": OHC,
        "idxs": idxs, "ohms": ohms,
    }


def _groups(K):
    """Split K chunks into gather groups; big groups then a small tail."""
    out = []
    c = 0
    while c < K:
        left = K - c
        if left > GCHUNK + 2 * TAILCH:
            n = GCHUNK
        elif left > 2 * TAILCH:
            n = left - 2 * TAILCH
        else:
            n = min(left, TAILCH)
        out.append((c, c + n))
        c += n
    return out


def _interleave(lens_by_stream, groups_by_stream):
    """Order gather groups by first consumption: walk slots, each stream's
    cursor advances by its per-slot length; append a stream's next group when
    the cursor first enters it."""
    order = []
    nxt = {k: 0 for k in groups_by_stream}
    cur = {k: 0 for k in groups_by_stream}
    for s in range(NSLOT):
        for name in STREAMS:
            cur[name] += int(lens_by_stream[name][s])
            g = groups_by_stream[name]
            while nxt[name] < len(g) and g[nxt[name]][0] * P < cur[name]:
                order.append((name, g[nxt[name]]))
                nxt[name] += 1
    for name in STREAMS:  # any remainder
        g = groups_by_stream[name]
        while nxt[name] < len(g):
            order.append((name, g[nxt[name]]))
            nxt[name] += 1
    return order


def pack_gidx(idx, groups):
    """[K*128] desc-position-major int16 indices -> [128, K*8] dma_gather
    layout (index i of a group at [i%16, i//16], replicated on partitions
    16..127)."""
    K = len(idx) // P
    out = np.zeros((P, K * 8), np.int16)
    for c0, c1 in groups:
        g = idx[c0 * P:c1 * P]
        blk = g.reshape(-1, 16).T
        out[:, c0 * 8:c1 * 8] = np.tile(blk, (8, 1))
    return out


# ---------------------------------------------------------------- program

def build(pl, dbg=False):
    import concourse.bass as bass
    import concourse.mybir as mybir
    from concourse import bacc
    import concourse.tile as tile

    bf16, f32, i16 = mybir.dt.bfloat16, mybir.dt.float32, mybir.dt.int16
    KA, KB, KS, OHC = pl["KA"], pl["KB"], pl["KS"], pl["OHC"]
    mms = pl["mms"]

    nc = bacc.Bacc("TRN2", debug=dbg, num_swdge_queues=NQUEUES)
    tabA = nc.dram_tensor("tabA", [NPAIR, 2 * P], bf16, kind="ExternalInput")
    tabB = nc.dram_tensor("tabB", [NPAIR, 2 * P], bf16, kind="ExternalInput")
    gxt = {name: nc.dram_tensor("gidx" + name, [P, k * 8], i16,
                                kind="ExternalInput")
           for name, k in (("A", KA), ("B", KB), ("S", KS))}
    ohm = nc.dram_tensor("ohm", [P, OHC], bf16, kind="ExternalInput")
    wmat = nc.dram_tensor("wmat", [P, P], bf16, kind="ExternalInput")
    bcol = nc.dram_tensor("bcol", [P, 1], f32, kind="ExternalInput")
    out = nc.dram_tensor("out", [P, NSLOT * P], f32, kind="ExternalOutput")

    groups = {name: _groups(k) for name, k in
              (("A", KA), ("B", KB), ("S", KS))}
    lens = {"A": pl["LA"], "B": pl["LB"], "S": pl["LSE"] + pl["LSO"]}
    gorder = _interleave(lens, groups)

    # oh slab column ranges per slot
    oh_rng = []
    for s in range(NSLOT):
        cols = [m[3] for m in mms[s]]
        oh_rng.append((min(cols) * P, (max(cols) + 1) * P))

    with tile.TileContext(nc) as tc:
        with tc.tile_pool(name="const", bufs=1) as cp, \
             tc.tile_pool(name="gA", bufs=GBUFS) as pA, \
             tc.tile_pool(name="gB", bufs=GBUFS) as pB, \
             tc.tile_pool(name="gS", bufs=GBUFS) as pS, \
             tc.tile_pool(name="ohp", bufs=6) as ohp, \
             tc.tile_pool(name="res", bufs=3) as resp, \
             tc.tile_pool(name="psA", bufs=4, space="PSUM") as psA, \
             tc.tile_pool(name="psB", bufs=2, space="PSUM") as psB:

            gidx_t = {nm: cp.tile([P, k * 8], i16, name="gidxt" + nm)
                      for nm, k in (("A", KA), ("B", KB), ("S", KS))}

            # first groups' indices land first so their gathers can issue
            first = gorder[:3]
            done = {"A": 0, "B": 0, "S": 0}
            for name, (c0, c1) in first:
                nc.sync.dma_start(out=gidx_t[name][:, c0 * 8:c1 * 8],
                                  in_=gxt[name][:, c0 * 8:c1 * 8])
                done[name] = max(done[name], c1)

            tabs = {"A": tabA, "B": tabB, "S": tabA}
            pools = {"A": pA, "B": pB, "S": pS}
            st = {name: {"tiles": {}, "g": 0} for name in STREAMS}
            qcount = [0]

            def fetch(name):
                S = st[name]
                gi = S["g"]
                c0, c1 = groups[name][gi]
                n = c1 - c0
                t = pools[name].tile([P, n * 2 * P], mybir.dt.bfloat16,
                                     tag="g" + name)
                nc.gpsimd.dma_gather(
                    out_ap=t[:].rearrange("p (g d) -> p g d", d=2 * P),
                    in_ap=tabs[name][:],
                    idxs_ap=gidx_t[name][:, c0 * 8:c1 * 8],
                    num_idxs=n * P,
                    num_idxs_reg=n * P,
                    elem_size=2 * P,
                    single_packet=SINGLE_PACKET,
                    queue_num=qcount[0] % NQUEUES,
                )
                qcount[0] += 1
                S["tiles"][gi] = (t, c0, c1)
                S["g"] += 1

            for name, _ in first:
                fetch(name)

            # remaining gidx in bulk, then weights/bias
            for name, K in (("A", KA), ("B", KB), ("S", KS)):
                c0 = done[name]
                if c0 < K:
                    nc.sync.dma_start(out=gidx_t[name][:, c0 * 8:K * 8],
                                      in_=gxt[name][:, c0 * 8:K * 8])
            w_t = cp.tile([P, P], bf16)
            nc.sync.dma_start(out=w_t[:], in_=wmat[:])
            b_t = cp.tile([P, 1], f32)
            nc.sync.dma_start(out=b_t[:], in_=bcol[:])

            # one-hot slabs per slot (pool depth throttles prefetch)
            oh_tiles = []
            for s in range(NSLOT):
                o0, o1 = oh_rng[s]
                t = ohp.tile([P, o1 - o0], bf16, tag="oh")
                nc.sync.dma_start(out=t[:], in_=ohm[:, o0:o1])
                oh_tiles.append((t, o0))

            for name, _ in gorder[3:]:
                fetch(name)

            def find_tile(name, j):
                S = st[name]
                for gi, (t, c0, c1) in S["tiles"].items():
                    if c0 <= j < c1:
                        return t, c0
                raise KeyError((name, j))

            for s in range(NSLOT):
                ps_agg = psA.tile([P, P], f32, tag="agg")
                oh_t, o0 = oh_tiles[s]
                n = len(mms[s])
                for k, (name, j, win, col) in enumerate(mms[s]):
                    t, c0 = find_tile(name, j)
                    lo = (j - c0) * 2 * P + win * P
                    oc = col * P - o0
                    nc.tensor.matmul(
                        out=ps_agg[:],
                        lhsT=t[:, lo:lo + P],
                        rhs=oh_t[:, oc:oc + P],
                        start=(k == 0), stop=(k == n - 1),
                    )
                aggT = resp.tile([P, P], mybir.dt.bfloat16, tag="aggT")
                nc.scalar.copy(out=aggT[:], in_=ps_agg[:])
                ps_out = psB.tile([P, P], f32, tag="out")
                nc.tensor.matmul(out=ps_out[:], lhsT=w_t[:], rhs=aggT[:],
                                 start=True, stop=True)
                o_sb = resp.tile([P, P], f32, tag="osb")
                nc.scalar.activation(
                    out=o_sb[:], in_=ps_out[:],
                    func=mybir.ActivationFunctionType.Identity,
                    bias=b_t[:, 0:1],
                )
                nc.sync.dma_start(out=out[:, s * P:(s + 1) * P], in_=o_sb[:])

    # Spread gathers across SWDGE queues (queue must be a function of the
    # scheduled DMASW lane; see baseline note).
    for inst in nc.inst_map.values():
        if isinstance(inst, mybir.InstDMAGatherAnt):
            proc = inst.bass_scheduled_proc
            if proc is not None and 11 <= proc <= 18:
                inst.queue_num = (proc - 11) % NQUEUES

    nc.compile()
    return nc


# ---------------------------------------------------------------- in_maps

def make_in_maps(features, W, b, pl):
    f16 = np.ascontiguousarray(features).astype(BF16)
    w_np = np.asarray(W, np.float32).astype(BF16)
    b_np = np.asarray(b, np.float32).reshape(1, D).T.copy()
    groups = {name: _groups(pl["K" + name]) for name in STREAMS}
    in_maps = []
    for c in range(NCORES):
        _, _, _, _, posA, posB = pl["per_core"][c]
        invA = np.empty(N_NODES, np.int64)
        invA[posA] = np.arange(N_NODES)
        invB = np.empty(N_NODES, np.int64)
        invB[posB] = np.arange(N_NODES)
        m = {
            "tabA": f16[invA].reshape(NPAIR, 2 * P),
            "tabB": f16[invB].reshape(NPAIR, 2 * P),
            "ohm": np.ascontiguousarray(pl["ohms"][c]).astype(BF16),
            "wmat": w_np,
            "bcol": b_np,
        }
        for name in STREAMS:
            m["gidx" + name] = pack_gidx(pl["idxs"][name][c], groups[name])
        in_maps.append(m)
    return in_maps


def unshard(outs, node_lists):
    full = np.zeros((N_NODES, D), np.float32)
    for c in range(NCORES):
        oT = np.asarray(outs[c]["out"], np.float32)
        for s in range(NSLOT):
            ns = node_lists[c][s]
            if len(ns) == 0:
                continue
            full[ns, :] = oT[:, s * P:s * P + len(ns)].T
    return full


# ---------------------------------------------------------------- entry

_CACHE = {}


def kernel(features, src, dst, W, b):
    from concourse.bass_utils import run_bass_kernel_spmd

    pl = plan(src, dst)
    key = (tuple(pl["LA"]), tuple(pl["LB"]),
           tuple(pl["LSE"]), tuple(pl["LSO"]))
    if key not in _CACHE:
        _CACHE[key] = build(pl)
    nc = _CACHE[key]
    in_maps = make_in_maps(features, W, b, pl)
    last = None
    for _ in range(3):  # retry: a previously wedged pool device can fail a load
        try:
            res = run_bass_kernel_spmd(nc, in_maps, core_ids=list(range(NCORES)))
            return unshard(res.results, pl["node_lists"])
        except Exception as e:  # noqa: BLE001
            last = e
    raise last
